# revision 15
# baseline (speedup 1.0000x reference)
"""Conv1d (B=32, C_in=C_out=256, W=4096, K=3, pad=1) on 8 Trainium2 cores.

Strategy: data-parallel over batch (4 per core). Per core the conv is 6
accumulated matmuls per 512-position output chunk: contraction over (tap u,
ci_chunk) with lhsT = weight tile [128 ci x 128 co] and rhs = a slice of
the padded-x row [128 ci x 512]. fp16 inputs, fp32 PSUM accumulation, bias
added during the PSUM->SBUF drain on DVE with fp16 output staging (halves
store bytes; host casts back to fp32).

v8 schedule, built on two measured facts: (1) each dma_start occupies its
issuing ring for ~1-1.5us regardless of size, (2) a single DMA's
descriptors spread across all 16 queues (~360 GB/s), so big DMAs are
almost as fast as small ones end-to-end. Hence:
- x is loaded as ONE DMA per (batch, ci chunk): [128, 4098] fp16 padded
  rows (8 loads total). Batch 0's pair goes first on the sync + gpsimd
  rings and lands ~4us after issue, so the whole batch is ready at once
  and the PE never starves mid-batch.
- weights are ONE [128, 12*128] DMA (+ small bias) on the scalar ring.
- a memset tile feeds 12 dummy matmuls emitted first so the tensor
  engine's DVFS ramp happens during the load latency.
- outputs are staged per (b, co) [128, 4096] fp16 rows; batches 0-2 store
  as whole rows, batch 3 per quarter with the final quarters split per
  512-chunk so the tail only waits on a 128KB transfer.
"""

import numpy as np

F16 = np.float16

B, C, W, K = 32, 256, 4096, 3
NCORES = 8
BPC = B // NCORES          # batches per core
P = 128                    # partitions
CIC = C // P               # ci chunks
COC = C // P               # co chunks
NCH = 512                  # positions per matmul (one PSUM bank of fp32)
NCHUNKS = W // NCH         # position chunks per batch row
XW = W + 2                 # padded x row width
QW = 1024                  # store quarter width
NWARM = 12                 # PE warm-up matmuls

_cache = {}


def _build_program():
    import concourse.bass as bass
    import concourse.bacc as bacc
    import concourse.mybir as mybir
    from concourse import tile

    nc = bacc.Bacc(None, target_bir_lowering=False)
    # x, padded by one zero position on each side: [b, ci_chunk, 128, 4098]
    x_d = nc.dram_tensor("xr", [BPC, CIC, P, XW], mybir.dt.float16,
                         kind="ExternalInput")
    # weight tiles side by side, t = coc*6 + u*CIC + cic, each [ci_in, co_in]
    w_d = nc.dram_tensor("wt", [P, K * CIC * COC * P], mybir.dt.float16,
                         kind="ExternalInput")
    b_d = nc.dram_tensor("bb", [P, COC], mybir.dt.float32,
                         kind="ExternalInput")
    out_d = nc.dram_tensor("out", [BPC, COC, P, W], mybir.dt.float16,
                           kind="ExternalOutput")

    with tile.TileContext(nc) as tc:
        with (
            tc.tile_pool(name="wp", bufs=3) as wp,
            tc.tile_pool(name="xpool", bufs=BPC * CIC) as xpool,
            tc.tile_pool(name="opool", bufs=5) as opool,
            tc.tile_pool(name="pspool", bufs=8, space=bass.MemorySpace.PSUM) as pspool,
        ):
            # -- PE warm-up ----------------------------------------------
            warm = wp.tile([P, NCH], mybir.dt.float16, name="warm", tag="warm")
            nc.gpsimd.memset(warm[:], 0)
            wps = pspool.tile([P, NCH], mybir.dt.float32, name="wps", tag="ps")
            for _ in range(NWARM):
                nc.tensor.matmul(wps[:], warm[:, 0:P], warm[:],
                                 start=True, stop=True)

            # -- tiles ----------------------------------------------------
            x_sb = {(b, ci): xpool.tile([P, XW], mybir.dt.float16,
                                        name=f"xr_{b}_{ci}", tag="xt")
                    for b in range(BPC) for ci in range(CIC)}
            w_sb = wp.tile([P, K * CIC * COC * P], mybir.dt.float16)
            b_sb = wp.tile([P, COC], mybir.dt.float32)

            # -- loads: one big DMA per (b, ci); b0 first on two rings ----
            for b in range(BPC):
                nc.sync.dma_start(x_sb[(b, 0)][:], x_d[b, 0])
                nc.gpsimd.dma_start(x_sb[(b, 1)][:], x_d[b, 1])
            nc.scalar.dma_start(w_sb[:], w_d[:])
            nc.scalar.dma_start(b_sb[:], b_d[:])

            # -- compute --------------------------------------------------
            NACC = K * CIC
            o_sb = {}
            for b in range(BPC):
                for co in range(COC):
                    o_sb[(b, co)] = None  # allocated at first chunk
                for n in range(NCHUNKS):
                    for co in range(COC):
                        if n == 0:
                            o_sb[(b, co)] = opool.tile(
                                [P, W], mybir.dt.float16,
                                name=f"or_{b}_{co}", tag="ot")
                        ps = pspool.tile([P, NCH], mybir.dt.float32,
                                         name=f"ps_{b}_{co}_{n}", tag="ps")
                        for k, (u, ci) in enumerate(
                                (u, ci) for u in range(K) for ci in range(CIC)):
                            nc.tensor.matmul(
                                ps[:],
                                w_sb[:, (co * NACC + u * CIC + ci) * P:
                                     (co * NACC + u * CIC + ci + 1) * P],
                                x_sb[(b, ci)][:, n * NCH + u:n * NCH + u + NCH],
                                start=(k == 0), stop=(k == NACC - 1),
                            )
                        nc.vector.tensor_scalar_add(
                            o_sb[(b, co)][:, n * NCH:(n + 1) * NCH], ps[:],
                            b_sb[:, co:co + 1],
                        )
                        # stores: whole rows for b0-b2 (scalar ring);
                        # b3 per quarter on sync, final quarters per chunk.
                        if b < BPC - 1:
                            if n == NCHUNKS - 1:
                                nc.scalar.dma_start(
                                    out_d[b, co], o_sb[(b, co)][:])
                        elif n >= 6:
                            nc.sync.dma_start(
                                out_d[b, co, :, n * NCH:(n + 1) * NCH],
                                o_sb[(b, co)][:, n * NCH:(n + 1) * NCH])
                        elif n % 2 == 1:
                            nc.sync.dma_start(
                                out_d[b, co, :, (n - 1) * NCH:(n + 1) * NCH],
                                o_sb[(b, co)][:, (n - 1) * NCH:(n + 1) * NCH])
    nc.compile()
    return nc


def _prep_inputs(x, weight, bias):
    # x: [32,256,4096] f32 -> padded fp16 rows [B, CIC, 128, 4098]
    xr = np.zeros((B, CIC, P, XW), F16)
    xr[:, :, :, 1:W + 1] = x.reshape(B, CIC, P, W).astype(F16)
    # weight: [co, ci, u] -> tiles [128, (coc*6 + u*CIC + cic)*128 + co_in]
    wt = weight.reshape(COC, P, CIC, P, K)          # [coc, co_in, cic, ci_in, u]
    w_host = np.ascontiguousarray(
        wt.transpose(3, 0, 4, 2, 1)                 # [ci_in, coc, u, cic, co_in]
    ).reshape(P, COC * K * CIC * P).astype(F16)
    b_host = np.ascontiguousarray(bias.reshape(COC, P).T).astype(np.float32)
    return xr, w_host, b_host


def run(x, weight, bias, trace=False):
    from concourse.bass_utils import run_bass_kernel_spmd

    if "nc" not in _cache:
        _cache["nc"] = _build_program()
    nc = _cache["nc"]

    xr, w_host, b_host = _prep_inputs(
        np.asarray(x, np.float32), np.asarray(weight, np.float32),
        np.asarray(bias, np.float32))
    in_maps = [
        {"xr": xr[c * BPC:(c + 1) * BPC], "wt": w_host, "bb": b_host}
        for c in range(NCORES)
    ]
    res = run_bass_kernel_spmd(nc, in_maps, list(range(NCORES)), trace=trace)
    out = np.concatenate(
        [res.results[c]["out"].reshape(BPC, C, W) for c in range(NCORES)],
        axis=0).astype(np.float32)
    return out, res


def kernel(x, weight, bias):
    out, _ = run(x, weight, bias, trace=False)
    return out


# revision 18
# speedup vs baseline: 1.0779x; 1.0779x over previous
"""Conv1d (B=32, C_in=C_out=256, W=4096, K=3, pad=1) on 8 Trainium2 cores.

Strategy: data-parallel over batch (4 per core). Per core the conv is 6
accumulated matmuls per 512-position output chunk: contraction over (tap u,
ci_chunk) with lhsT = weight tile [128 ci x 128 co] and rhs = a slice of
the padded-x row [128 ci x 512]. fp16 inputs, fp32 PSUM accumulation, bias
added during the PSUM->SBUF drain on DVE with fp16 output staging (halves
store bytes; host casts back to fp32).

v8 schedule, built on two measured facts: (1) each dma_start occupies its
issuing ring for ~1-1.5us regardless of size, (2) a single DMA's
descriptors spread across all 16 queues (~360 GB/s), so big DMAs are
almost as fast as small ones end-to-end. Hence:
- x is loaded as ONE DMA per (batch, ci chunk): [128, 4098] fp16 padded
  rows (8 loads total). Batch 0's pair goes first on the sync + gpsimd
  rings and lands ~4us after issue, so the whole batch is ready at once
  and the PE never starves mid-batch.
- weights are ONE [128, 12*128] DMA (+ small bias) on the scalar ring.
- a memset tile feeds 12 dummy matmuls emitted first so the tensor
  engine's DVFS ramp happens during the load latency.
- outputs are staged per (b, co) [128, 4096] fp16 rows; batches 0-2 store
  as whole rows, batch 3 per quarter with the final quarters split per
  512-chunk so the tail only waits on a 128KB transfer.
"""

import numpy as np

F16 = np.float16

B, C, W, K = 32, 256, 4096, 3
NCORES = 8
BPC = B // NCORES          # batches per core
P = 128                    # partitions
CIC = C // P               # ci chunks
COC = C // P               # co chunks
NCH = 512                  # positions per matmul (one PSUM bank of fp32)
NCHUNKS = W // NCH         # position chunks per batch row
XW = W + 2                 # padded x row width
QW = 1024                  # store quarter width
NWARM = 12                 # PE warm-up matmuls

_cache = {}


def _build_program():
    import concourse.bass as bass
    import concourse.bacc as bacc
    import concourse.mybir as mybir
    from concourse import tile

    nc = bacc.Bacc(None, target_bir_lowering=False)
    # x, padded by one zero position on each side: [b, ci_chunk, 128, 4098]
    x_d = nc.dram_tensor("xr", [BPC, CIC, P, XW], mybir.dt.float16,
                         kind="ExternalInput")
    # weight tiles side by side, t = coc*6 + u*CIC + cic, each [ci_in, co_in]
    w_d = nc.dram_tensor("wt", [P, K * CIC * COC * P], mybir.dt.float16,
                         kind="ExternalInput")
    b_d = nc.dram_tensor("bb", [P, COC], mybir.dt.float32,
                         kind="ExternalInput")
    out_d = nc.dram_tensor("out", [BPC, COC, P, W], mybir.dt.float16,
                           kind="ExternalOutput")

    with tile.TileContext(nc) as tc:
        with (
            tc.tile_pool(name="wp", bufs=3) as wp,
            tc.tile_pool(name="xpool", bufs=(BPC - 1) * CIC + CIC * 4) as xpool,
            tc.tile_pool(name="opool", bufs=5) as opool,
            tc.tile_pool(name="pspool", bufs=8, space=bass.MemorySpace.PSUM) as pspool,
        ):
            # -- PE warm-up ----------------------------------------------
            warm = wp.tile([P, NCH], mybir.dt.float16, name="warm", tag="warm")
            nc.gpsimd.memset(warm[:], 0)
            wps = pspool.tile([P, NCH], mybir.dt.float32, name="wps", tag="ps")
            for _ in range(NWARM):
                nc.tensor.matmul(wps[:], warm[:, 0:P], warm[:],
                                 start=True, stop=True)

            # -- tiles ----------------------------------------------------
            # batch 0 arrives as quarter tiles [128, 1026] (fast first
            # load); batches 1-3 as whole padded rows (one ring slot each).
            xq_sb = {(ci, qq): xpool.tile([P, QW + 2], mybir.dt.float16,
                                          name=f"xq_{ci}_{qq}", tag="xt")
                     for ci in range(CIC) for qq in range(4)}
            x_sb = {(b, ci): xpool.tile([P, XW], mybir.dt.float16,
                                        name=f"xr_{b}_{ci}", tag="xt")
                    for b in range(1, BPC) for ci in range(CIC)}
            w_sb = wp.tile([P, K * CIC * COC * P], mybir.dt.float16)
            b_sb = wp.tile([P, COC], mybir.dt.float32)

            # -- loads: b0 quarters then whole rows, split over two rings -
            for qq in range(4):
                nc.sync.dma_start(xq_sb[(0, qq)][:],
                                  x_d[0, 0][:, qq * QW:qq * QW + QW + 2])
                nc.gpsimd.dma_start(xq_sb[(1, qq)][:],
                                    x_d[0, 1][:, qq * QW:qq * QW + QW + 2])
            for b in range(1, BPC):
                nc.sync.dma_start(x_sb[(b, 0)][:], x_d[b, 0])
                nc.gpsimd.dma_start(x_sb[(b, 1)][:], x_d[b, 1])
            nc.scalar.dma_start(w_sb[:], w_d[:])
            nc.scalar.dma_start(b_sb[:], b_d[:])

            # -- compute --------------------------------------------------
            NACC = K * CIC
            o_sb = {}
            for b in range(BPC):
                for co in range(COC):
                    o_sb[(b, co)] = None  # allocated at first chunk
                for n in range(NCHUNKS):
                    for co in range(COC):
                        if n == 0:
                            o_sb[(b, co)] = opool.tile(
                                [P, W], mybir.dt.float16,
                                name=f"or_{b}_{co}", tag="ot")
                        ps = pspool.tile([P, NCH], mybir.dt.float32,
                                         name=f"ps_{b}_{co}_{n}", tag="ps")
                        for k, (u, ci) in enumerate(
                                (u, ci) for u in range(K) for ci in range(CIC)):
                            if b == 0:
                                rhs = xq_sb[(ci, n // 2)][
                                    :, (n % 2) * NCH + u:(n % 2) * NCH + u + NCH]
                            else:
                                rhs = x_sb[(b, ci)][
                                    :, n * NCH + u:n * NCH + u + NCH]
                            nc.tensor.matmul(
                                ps[:],
                                w_sb[:, (co * NACC + u * CIC + ci) * P:
                                     (co * NACC + u * CIC + ci + 1) * P],
                                rhs,
                                start=(k == 0), stop=(k == NACC - 1),
                            )
                        nc.vector.tensor_scalar_add(
                            o_sb[(b, co)][:, n * NCH:(n + 1) * NCH], ps[:],
                            b_sb[:, co:co + 1],
                        )
                        # stores: whole rows for b0-b2 (scalar ring);
                        # b3 per quarter on sync, final quarters per chunk.
                        if b < BPC - 1:
                            if n == NCHUNKS - 1:
                                nc.scalar.dma_start(
                                    out_d[b, co], o_sb[(b, co)][:])
                        elif n >= 6:
                            nc.sync.dma_start(
                                out_d[b, co, :, n * NCH:(n + 1) * NCH],
                                o_sb[(b, co)][:, n * NCH:(n + 1) * NCH])
                        elif n % 2 == 1:
                            nc.sync.dma_start(
                                out_d[b, co, :, (n - 1) * NCH:(n + 1) * NCH],
                                o_sb[(b, co)][:, (n - 1) * NCH:(n + 1) * NCH])
    nc.compile()
    return nc


def _prep_inputs(x, weight, bias):
    # x: [32,256,4096] f32 -> padded fp16 rows [B, CIC, 128, 4098]
    xr = np.zeros((B, CIC, P, XW), F16)
    xr[:, :, :, 1:W + 1] = x.reshape(B, CIC, P, W).astype(F16)
    # weight: [co, ci, u] -> tiles [128, (coc*6 + u*CIC + cic)*128 + co_in]
    wt = weight.reshape(COC, P, CIC, P, K)          # [coc, co_in, cic, ci_in, u]
    w_host = np.ascontiguousarray(
        wt.transpose(3, 0, 4, 2, 1)                 # [ci_in, coc, u, cic, co_in]
    ).reshape(P, COC * K * CIC * P).astype(F16)
    b_host = np.ascontiguousarray(bias.reshape(COC, P).T).astype(np.float32)
    return xr, w_host, b_host


def run(x, weight, bias, trace=False):
    from concourse.bass_utils import run_bass_kernel_spmd

    if "nc" not in _cache:
        _cache["nc"] = _build_program()
    nc = _cache["nc"]

    xr, w_host, b_host = _prep_inputs(
        np.asarray(x, np.float32), np.asarray(weight, np.float32),
        np.asarray(bias, np.float32))
    in_maps = [
        {"xr": xr[c * BPC:(c + 1) * BPC], "wt": w_host, "bb": b_host}
        for c in range(NCORES)
    ]
    res = run_bass_kernel_spmd(nc, in_maps, list(range(NCORES)), trace=trace)
    out = np.concatenate(
        [res.results[c]["out"].reshape(BPC, C, W) for c in range(NCORES)],
        axis=0).astype(np.float32)
    return out, res


def kernel(x, weight, bias):
    out, _ = run(x, weight, bias, trace=False)
    return out
